# revision 13
# baseline (speedup 1.0000x reference)
"""Causal self-attention (B=4, S=2048, D=1024, H=16) on 8 trn2 NeuronCores.

Sharding: core c handles batch b = c//2 and heads h0 = (c%2)*8 .. h0+8
(data parallel over batch x tensor parallel over head halves).

Per-core device kernel (feature-major S^T formulation, no on-device
transposes, matmuls in float32r for full PE rate):
  Q^T = (w_q/8)^T x  + b_q/8     [512, 2048]  (scale folded into weights)
  K^T = w_k^T x                  [512, 2048]  (b_k dropped: softmax shift inv.)
  V   = x w_v                    [2048, 512]  (b_v handled on host: P @ 1 = 1)
  K^T is stored per head zero-padded to 128 feature rows (KTh) so the
  S^T matmuls contract over K=128 - K=64 matmuls run at half clock and
  never warm the PE HAM clock gate.
  per head, per k-tile kt: S^T[k,q] = KTh.T @ Q^T  (one matmul per 512 q)
    exp on ACT (no max-subtraction: |scores| <~ 2), then multiplicative
    0/1 causal mask on the diagonal-band 512 block
  O'^T[d,q] accumulated over kt: lhsT = [V_h | ones] (M=65) -> row 64 gives
    softmax row-sums for free; normalize O' columns by 1/rowsum
  Y_part = O'^T.T @ w_proj rows  [2048, 1024]
Host: y[b] = part(core 2b) + part(core 2b+1) + b_proj + b_v @ w_proj.
"""

import numpy as np

D_MODEL = 1024
N_HEADS = 16
HEAD_DIM = 64
B = 4
S = 2048
HPC = 8          # heads per core
CORES = 8
FPC = HPC * HEAD_DIM  # 512 features per core

_CACHE = {}


def _build():
    import concourse.bacc as bacc
    import concourse.tile as tile
    import concourse.mybir as mybir

    f32 = mybir.dt.float32
    f32r = mybir.dt.float32r
    Exp = mybir.ActivationFunctionType.Exp

    nc = bacc.Bacc("TRN2", debug=False)
    xT = nc.dram_tensor("xT", [D_MODEL, S], f32r, kind="ExternalInput").ap()
    wq = nc.dram_tensor("wq", [D_MODEL, FPC], f32r, kind="ExternalInput").ap()
    wk = nc.dram_tensor("wk", [D_MODEL, FPC], f32r, kind="ExternalInput").ap()
    wv = nc.dram_tensor("wv", [D_MODEL, FPC], f32r, kind="ExternalInput").ap()
    bq = nc.dram_tensor("bq", [FPC, 1], f32, kind="ExternalInput").ap()
    wp = nc.dram_tensor("wp", [FPC, D_MODEL], f32r, kind="ExternalInput").ap()
    y = nc.dram_tensor("y", [S, D_MODEL], f32, kind="ExternalOutput").ap()

    NT = S // 128        # 16 token tiles
    NQC = S // 512       # 4 q-chunks of 512
    KCH = D_MODEL // 128  # 8 contraction chunks

    with tile.TileContext(nc) as tc:
        with tc.tile_pool(name="persist", bufs=1) as persist:
            QT = [persist.tile([128, S], f32r, name=f"QT{p}") for p in range(4)]
            # K^T per head, zero-padded to 128 feature rows
            KTh = [persist.tile([128, S], f32r, name=f"KTh{h}") for h in range(HPC)]
            # V tiles augmented with a ones column per head: [128, 8*65]
            Vb = [persist.tile([128, HPC * 65], f32r, name=f"Vb{k}")
                  for k in range(NT)]
            bq_sb = persist.tile([128, 4], f32, name="bq_sb")
            for m in range(4):
                nc.sync.dma_start(out=bq_sb[:, m:m + 1],
                                  in_=bq[m * 128:(m + 1) * 128, :])
            for kt in range(NT):
                ones_col = Vb[kt].rearrange("p (h d) -> p h d", d=65)[:, :, 64:65]
                nc.gpsimd.memset(ones_col.bitcast(f32), 1.0)
            for h in range(HPC):
                dead = (slice(64, 128) if h % 2 == 0 else slice(0, 64))
                nc.gpsimd.memset(KTh[h][dead, :].bitcast(f32), 0.0)

            # ---------------- Phase A: projections (V, then Q, then K) ----
            with tc.tile_pool(name="xpool", bufs=2) as xpool:
                with tc.tile_pool(name="wvpool", bufs=1) as wvpool, \
                     tc.tile_pool(name="psV", bufs=3, space="PSUM") as psV:
                    wv_sb = [wvpool.tile([128, FPC], f32r, name=f"wvs{k}")
                             for k in range(KCH)]
                    for k in range(KCH):
                        nc.sync.dma_start(out=wv_sb[k],
                                          in_=wv[k * 128:(k + 1) * 128, :])
                    for tch in range(NQC):
                        xk = xpool.tile([128, KCH, 512], f32r, name="xk", tag="xk")
                        for k in range(KCH):
                            nc.sync.dma_start(
                                out=xk[:, k, :],
                                in_=xT[k * 128:(k + 1) * 128,
                                       tch * 512:(tch + 1) * 512])
                        for tt in range(4):
                            psv = psV.tile([128, 512], f32, name="psv", tag="psv")
                            for k in range(KCH):
                                nc.tensor.matmul(
                                    psv, lhsT=xk[:, k, tt * 128:(tt + 1) * 128],
                                    rhs=wv_sb[k],
                                    start=(k == 0), stop=(k == KCH - 1))
                            kt = tch * 4 + tt
                            nc.vector.tensor_copy(
                                out=Vb[kt].rearrange("p (h d) -> p h d", d=65)[:, :, 0:64],
                                in_=psv.rearrange("p (h d) -> p h d", d=64))

                with tc.tile_pool(name="wqpool", bufs=1) as wqpool, \
                     tc.tile_pool(name="psQ", bufs=3, space="PSUM") as psQ:
                    wq_sb = [wqpool.tile([128, FPC], f32r, name=f"wqs{k}")
                             for k in range(KCH)]
                    for k in range(KCH):
                        nc.sync.dma_start(out=wq_sb[k],
                                          in_=wq[k * 128:(k + 1) * 128, :])
                    for tch in range(NQC):
                        xk = xpool.tile([128, KCH, 512], f32r, name="xk", tag="xk")
                        for k in range(KCH):
                            nc.sync.dma_start(
                                out=xk[:, k, :],
                                in_=xT[k * 128:(k + 1) * 128,
                                       tch * 512:(tch + 1) * 512])
                        for m in range(4):
                            psq = psQ.tile([128, 512], f32, name="psq", tag="psq")
                            for k in range(KCH):
                                nc.tensor.matmul(
                                    psq, lhsT=wq_sb[k][:, m * 128:(m + 1) * 128],
                                    rhs=xk[:, k, :],
                                    start=(k == 0), stop=(k == KCH - 1))
                            nc.vector.tensor_scalar_add(
                                QT[m][:, tch * 512:(tch + 1) * 512], psq,
                                bq_sb[:, m:m + 1])

                with tc.tile_pool(name="wkpool", bufs=1) as wkpool, \
                     tc.tile_pool(name="psK", bufs=3, space="PSUM") as psK:
                    wk_sb = [wkpool.tile([128, FPC], f32r, name=f"wks{k}")
                             for k in range(KCH)]
                    for k in range(KCH):
                        nc.sync.dma_start(out=wk_sb[k],
                                          in_=wk[k * 128:(k + 1) * 128, :])
                    for tch in range(NQC):
                        xk = xpool.tile([128, KCH, 512], f32r, name="xk", tag="xk")
                        for k in range(KCH):
                            nc.sync.dma_start(
                                out=xk[:, k, :],
                                in_=xT[k * 128:(k + 1) * 128,
                                       tch * 512:(tch + 1) * 512])
                        for m in range(4):
                            psk = psK.tile([128, 512], f32, name="psk", tag="psk")
                            for k in range(KCH):
                                nc.tensor.matmul(
                                    psk, lhsT=wk_sb[k][:, m * 128:(m + 1) * 128],
                                    rhs=xk[:, k, :],
                                    start=(k == 0), stop=(k == KCH - 1))
                            sl = slice(tch * 512, (tch + 1) * 512)
                            nc.vector.tensor_copy(
                                out=KTh[2 * m][0:64, sl], in_=psk[0:64, :])
                            nc.vector.tensor_copy(
                                out=KTh[2 * m + 1][64:128, sl], in_=psk[64:128, :])

            # ---------------- OT (persists into phase C) ----------------
            with tc.tile_pool(name="opool", bufs=1) as opool:
                OT = [opool.tile([128, S], f32r, name=f"OT{p}") for p in range(4)]
                wp_sb = [opool.tile([128, D_MODEL], f32r, name=f"wps{p}")
                         for p in range(4)]
                for p in range(4):
                    nc.sync.dma_start(out=wp_sb[p],
                                      in_=wp[p * 128:(p + 1) * 128, :])

                # ---------------- Phase B: attention ----------------
                with tc.tile_pool(name="bpool", bufs=1) as bpool, \
                     tc.tile_pool(name="ptpool", bufs=2) as ptpool, \
                     tc.tile_pool(name="work", bufs=2) as work:
                    # additive causal triangle mask: 0 where col >= row
                    tri = bpool.tile([128, 128], f32, name="tri")
                    nc.gpsimd.memset(tri, 0.0)
                    nc.gpsimd.affine_select(
                        out=tri, in_=tri,
                        compare_op=mybir.AluOpType.is_ge, fill=-1e5,
                        base=0, pattern=[[1, 128]], channel_multiplier=-1)

                    with tc.tile_pool(name="pss", bufs=2, space="PSUM") as pss, \
                         tc.tile_pool(name="pso", bufs=1, space="PSUM") as pso:
                        for head in range(HPC):
                            p_idx, part = head // 2, (head % 2) * 64
                            pso_t = [pso.tile([128, 512], f32, name=f"pso{qc}",
                                              tag=f"pso{qc}") for qc in range(NQC)]
                            for kt in range(NT):
                                k0 = kt * 128
                                ext = S - k0
                                pt = ptpool.tile([128, S], f32r, name="pt", tag="pt")
                                off = 0
                                while off < ext:
                                    psz = min(1024, ext - off)
                                    pss_t = pss.tile([128, 1024], f32,
                                                     name="pss_t", tag="pss_t")
                                    so = 0
                                    while so < psz:
                                        sz = min(512, psz - so)
                                        nc.tensor.matmul(
                                            pss_t[:, so:so + sz],
                                            lhsT=KTh[head][:, k0:k0 + 128],
                                            rhs=QT[p_idx][:, k0 + off + so:
                                                          k0 + off + so + sz],
                                            start=True, stop=True)
                                        so += 512
                                    if off == 0:
                                        nc.vector.tensor_add(
                                            pss_t[:, 0:128], pss_t[:, 0:128],
                                            tri)
                                    nc.scalar.activation(
                                        out=pt[:, off:off + psz],
                                        in_=pss_t[:, 0:psz], func=Exp)
                                    off += 1024
                                for qc in range(kt // 4, NQC):
                                    q_lo = max(qc * 512, k0)
                                    n = (qc + 1) * 512 - q_lo
                                    nc.tensor.matmul(
                                        pso_t[qc][0:65, q_lo - qc * 512:512],
                                        lhsT=Vb[kt][:, head * 65:head * 65 + 65],
                                        rhs=pt[:, q_lo - k0:q_lo - k0 + n],
                                        start=(kt == 0), stop=(kt == qc * 4 + 3))
                            for qc in range(NQC):
                                rs0 = work.tile([1, 512], f32, name="rs0", tag="rs0", bufs=1)
                                nc.vector.tensor_copy(rs0, pso_t[qc][64:65, :])
                                rs = work.tile([1, 512], f32, name="rs", tag="rs", bufs=1)
                                nc.vector.reciprocal_approx_fast(out=rs, in_=rs0[0:1, :])
                                rc = work.tile([64, 512], f32, name="rc", tag="rc")
                                nc.gpsimd.partition_broadcast(rc, rs[0:1, :])
                                if part == 0:
                                    nc.vector.tensor_mul(
                                        OT[p_idx][0:64, qc * 512:(qc + 1) * 512],
                                        pso_t[qc][0:64, :], rc)
                                else:
                                    tmp = work.tile([64, 512], f32r, name="tmp", tag="tmp")
                                    nc.vector.tensor_mul(tmp, pso_t[qc][0:64, :], rc)
                                    nc.sync.dma_start(
                                        out=OT[p_idx][64:128, qc * 512:(qc + 1) * 512],
                                        in_=tmp)

                # ---------------- Phase C: output projection ----------------
                with tc.tile_pool(name="psC", bufs=2, space="PSUM") as psC, \
                     tc.tile_pool(name="ypool", bufs=3) as ypool:
                    for tt in range(NT):
                        for half in range(2):
                            psy = psC.tile([128, 512], f32, name="psy", tag="psy")
                            for p in range(4):
                                nc.tensor.matmul(
                                    psy, lhsT=OT[p][:, tt * 128:(tt + 1) * 128],
                                    rhs=wp_sb[p][:, half * 512:(half + 1) * 512],
                                    start=(p == 0), stop=(p == 3))
                            ysb = ypool.tile([128, 512], f32, name="ysb", tag="ysb")
                            nc.vector.tensor_copy(ysb, psy)
                            nc.sync.dma_start(
                                out=y[tt * 128:(tt + 1) * 128,
                                      half * 512:(half + 1) * 512], in_=ysb)

    nc.compile()
    return nc


def _get_nc():
    if "nc" not in _CACHE:
        _CACHE["nc"] = _build()
    return _CACHE["nc"]


def _make_in_maps(x, w_qkv, b_qkv, w_proj):
    scale = np.float32(1.0 / np.sqrt(HEAD_DIM))
    in_maps = []
    for c in range(CORES):
        b = c // 2
        h0 = (c % 2) * HPC
        f0 = h0 * HEAD_DIM
        in_maps.append({
            "xT": np.ascontiguousarray(x[b].T),
            "wq": np.ascontiguousarray(w_qkv[:, f0:f0 + FPC] * scale),
            "wk": np.ascontiguousarray(w_qkv[:, D_MODEL + f0:D_MODEL + f0 + FPC]),
            "wv": np.ascontiguousarray(w_qkv[:, 2 * D_MODEL + f0:2 * D_MODEL + f0 + FPC]),
            "bq": np.ascontiguousarray(
                (b_qkv[f0:f0 + FPC] * scale).reshape(FPC, 1)),
            "wp": np.ascontiguousarray(w_proj[f0:f0 + FPC, :]),
        })
    return in_maps


def kernel(x, w_qkv, b_qkv, w_proj, b_proj):
    from concourse.bass_utils import run_bass_kernel_spmd

    x = np.asarray(x, np.float32)
    w_qkv = np.asarray(w_qkv, np.float32)
    b_qkv = np.asarray(b_qkv, np.float32)
    w_proj = np.asarray(w_proj, np.float32)
    b_proj = np.asarray(b_proj, np.float32)

    nc = _get_nc()
    in_maps = _make_in_maps(x, w_qkv, b_qkv, w_proj)
    res = run_bass_kernel_spmd(nc, in_maps, core_ids=list(range(CORES)))

    # host-side bias corrections: b_proj plus b_v routed through w_proj
    # (softmax rows sum to one, so P @ (1 b_v^T) W_p = 1 (b_v^T W_p))
    bv = b_qkv[2 * D_MODEL:3 * D_MODEL]
    bias_row = (bv @ w_proj + b_proj).astype(np.float32)

    out = np.empty((B, S, D_MODEL), np.float32)
    for b in range(B):
        out[b] = res.results[2 * b]["y"] + res.results[2 * b + 1]["y"] + bias_row
    return out


# revision 14
# speedup vs baseline: 1.2138x; 1.2138x over previous
"""Causal self-attention (B=4, S=2048, D=1024, H=16) on 8 trn2 NeuronCores.

Sharding: core c handles batch b = c//2 and heads h0 = (c%2)*8 .. h0+8
(data parallel over batch x tensor parallel over head halves).

Per-core device kernel (feature-major S^T formulation, no on-device
transposes, matmuls in float32r for full PE rate):
  Q^T = (w_q/8)^T x  + b_q/8     [512, 2048]  (scale folded into weights)
  K^T = w_k^T x                  [512, 2048]  (b_k dropped: softmax shift inv.)
  V   = x w_v                    [2048, 512]  (b_v handled on host: P @ 1 = 1)
  K^T is stored per head zero-padded to 128 feature rows (KTh) so the
  S^T matmuls contract over K=128 - K=64 matmuls run at half clock and
  never warm the PE HAM clock gate.
  per head, per k-tile kt: S^T[k,q] = KTh.T @ Q^T  (one matmul per 512 q)
    exp on ACT (no max-subtraction: |scores| <~ 2), then multiplicative
    0/1 causal mask on the diagonal-band 512 block
  O'^T[d,q] accumulated over kt: lhsT = [V_h | ones] (M=65) -> row 64 gives
    softmax row-sums for free; normalize O' columns by 1/rowsum
  Y_part = O'^T.T @ w_proj rows  [2048, 1024]
Host: y[b] = part(core 2b) + part(core 2b+1) + b_proj + b_v @ w_proj.
"""

import numpy as np

D_MODEL = 1024
N_HEADS = 16
HEAD_DIM = 64
B = 4
S = 2048
HPC = 8          # heads per core
CORES = 8
FPC = HPC * HEAD_DIM  # 512 features per core

_CACHE = {}


def _build():
    import concourse.bacc as bacc
    import concourse.tile as tile
    import concourse.mybir as mybir

    f32 = mybir.dt.float32
    f32r = mybir.dt.float32r
    Exp = mybir.ActivationFunctionType.Exp

    nc = bacc.Bacc("TRN2", debug=False)
    xT = nc.dram_tensor("xT", [D_MODEL, S], f32r, kind="ExternalInput").ap()
    wq = nc.dram_tensor("wq", [D_MODEL, FPC], f32r, kind="ExternalInput").ap()
    wk = nc.dram_tensor("wk", [D_MODEL, FPC], f32r, kind="ExternalInput").ap()
    wv = nc.dram_tensor("wv", [D_MODEL, FPC], f32r, kind="ExternalInput").ap()
    bq = nc.dram_tensor("bq", [FPC, 1], f32, kind="ExternalInput").ap()
    wp = nc.dram_tensor("wp", [FPC, D_MODEL], f32r, kind="ExternalInput").ap()
    y = nc.dram_tensor("y", [S, D_MODEL], f32, kind="ExternalOutput").ap()

    NT = S // 128        # 16 token tiles
    NQC = S // 512       # 4 q-chunks of 512
    KCH = D_MODEL // 128  # 8 contraction chunks

    with tile.TileContext(nc) as tc:
        with tc.tile_pool(name="persist", bufs=1) as persist:
            QT = [persist.tile([128, S], f32r, name=f"QT{p}") for p in range(4)]
            # K^T per head, zero-padded to 128 feature rows
            KTh = [persist.tile([128, S], f32r, name=f"KTh{h}") for h in range(HPC)]
            # V tiles augmented with a ones column per head: [128, 8*65]
            Vb = [persist.tile([128, HPC * 65], f32r, name=f"Vb{k}")
                  for k in range(NT)]
            bq_sb = persist.tile([128, 4], f32, name="bq_sb")
            for m in range(4):
                nc.sync.dma_start(out=bq_sb[:, m:m + 1],
                                  in_=bq[m * 128:(m + 1) * 128, :])
            for kt in range(NT):
                ones_col = Vb[kt].rearrange("p (h d) -> p h d", d=65)[:, :, 64:65]
                nc.gpsimd.memset(ones_col.bitcast(f32), 1.0)
            for h in range(HPC):
                dead = (slice(64, 128) if h % 2 == 0 else slice(0, 64))
                nc.gpsimd.memset(KTh[h][dead, :].bitcast(f32), 0.0)

            # ---------------- Phase A: projections (V, then Q, then K) ----
            with tc.tile_pool(name="xpool", bufs=2) as xpool:
                with tc.tile_pool(name="wvpool", bufs=1) as wvpool, \
                     tc.tile_pool(name="psV", bufs=3, space="PSUM") as psV:
                    wv_sb = [wvpool.tile([128, FPC], f32r, name=f"wvs{k}")
                             for k in range(KCH)]
                    for k in range(KCH):
                        nc.sync.dma_start(out=wv_sb[k],
                                          in_=wv[k * 128:(k + 1) * 128, :])
                    for tch in range(NQC):
                        xk = xpool.tile([128, KCH, 512], f32r, name="xk", tag="xk")
                        for k in range(KCH):
                            nc.sync.dma_start(
                                out=xk[:, k, :],
                                in_=xT[k * 128:(k + 1) * 128,
                                       tch * 512:(tch + 1) * 512])
                        for tt in range(4):
                            psv = psV.tile([128, 512], f32, name="psv", tag="psv")
                            for k in range(KCH):
                                nc.tensor.matmul(
                                    psv, lhsT=xk[:, k, tt * 128:(tt + 1) * 128],
                                    rhs=wv_sb[k],
                                    start=(k == 0), stop=(k == KCH - 1))
                            kt = tch * 4 + tt
                            nc.vector.tensor_copy(
                                out=Vb[kt].rearrange("p (h d) -> p h d", d=65)[:, :, 0:64],
                                in_=psv.rearrange("p (h d) -> p h d", d=64))

                with tc.tile_pool(name="wqpool", bufs=1) as wqpool, \
                     tc.tile_pool(name="psQ", bufs=3, space="PSUM") as psQ:
                    wq_sb = [wqpool.tile([128, FPC], f32r, name=f"wqs{k}")
                             for k in range(KCH)]
                    for k in range(KCH):
                        nc.sync.dma_start(out=wq_sb[k],
                                          in_=wq[k * 128:(k + 1) * 128, :])
                    for tch in range(NQC):
                        xk = xpool.tile([128, KCH, 512], f32r, name="xk", tag="xk")
                        for k in range(KCH):
                            nc.sync.dma_start(
                                out=xk[:, k, :],
                                in_=xT[k * 128:(k + 1) * 128,
                                       tch * 512:(tch + 1) * 512])
                        for m in range(4):
                            psq = psQ.tile([128, 512], f32, name="psq", tag="psq")
                            for k in range(KCH):
                                nc.tensor.matmul(
                                    psq, lhsT=wq_sb[k][:, m * 128:(m + 1) * 128],
                                    rhs=xk[:, k, :],
                                    start=(k == 0), stop=(k == KCH - 1))
                            nc.vector.tensor_scalar_add(
                                QT[m][:, tch * 512:(tch + 1) * 512], psq,
                                bq_sb[:, m:m + 1])

                with tc.tile_pool(name="wkpool", bufs=1) as wkpool, \
                     tc.tile_pool(name="psK", bufs=3, space="PSUM") as psK:
                    wk_sb = [wkpool.tile([128, FPC], f32r, name=f"wks{k}")
                             for k in range(KCH)]
                    for k in range(KCH):
                        nc.sync.dma_start(out=wk_sb[k],
                                          in_=wk[k * 128:(k + 1) * 128, :])
                    for tch in range(NQC):
                        xk = xpool.tile([128, KCH, 512], f32r, name="xk", tag="xk")
                        for k in range(KCH):
                            nc.sync.dma_start(
                                out=xk[:, k, :],
                                in_=xT[k * 128:(k + 1) * 128,
                                       tch * 512:(tch + 1) * 512])
                        for m in range(4):
                            psk = psK.tile([128, 512], f32, name="psk", tag="psk")
                            for k in range(KCH):
                                nc.tensor.matmul(
                                    psk, lhsT=wk_sb[k][:, m * 128:(m + 1) * 128],
                                    rhs=xk[:, k, :],
                                    start=(k == 0), stop=(k == KCH - 1))
                            sl = slice(tch * 512, (tch + 1) * 512)
                            nc.vector.tensor_copy(
                                out=KTh[2 * m][0:64, sl], in_=psk[0:64, :])
                            nc.vector.tensor_copy(
                                out=KTh[2 * m + 1][64:128, sl], in_=psk[64:128, :])

            # ---------------- OT (persists into phase C) ----------------
            with tc.tile_pool(name="opool", bufs=1) as opool:
                OT = [opool.tile([128, S], f32r, name=f"OT{p}") for p in range(4)]
                wp_sb = [opool.tile([128, D_MODEL], f32r, name=f"wps{p}")
                         for p in range(4)]
                for p in range(4):
                    nc.sync.dma_start(out=wp_sb[p],
                                      in_=wp[p * 128:(p + 1) * 128, :])

                # ---------------- Phase B: attention ----------------
                with tc.tile_pool(name="bpool", bufs=1) as bpool, \
                     tc.tile_pool(name="ptpool", bufs=2) as ptpool, \
                     tc.tile_pool(name="work", bufs=2) as work:
                    # additive causal triangle mask: 0 where col >= row
                    tri = bpool.tile([128, 128], f32, name="tri")
                    nc.gpsimd.memset(tri, 0.0)
                    nc.gpsimd.affine_select(
                        out=tri, in_=tri,
                        compare_op=mybir.AluOpType.is_ge, fill=-1e5,
                        base=0, pattern=[[1, 128]], channel_multiplier=-1)

                    with tc.tile_pool(name="pss", bufs=4, space="PSUM") as pss, \
                         tc.tile_pool(name="pso", bufs=1, space="PSUM") as pso:
                        for head in range(HPC):
                            p_idx, part = head // 2, (head % 2) * 64
                            pso_t = [pso.tile([128, 512], f32, name=f"pso{qc}",
                                              tag=f"pso{qc}") for qc in range(NQC)]
                            for kt in range(NT):
                                k0 = kt * 128
                                ext = S - k0
                                pt = ptpool.tile([128, S], f32r, name="pt", tag="pt")
                                off = 0
                                while off < ext:
                                    sz = min(512, ext - off)
                                    pss_t = pss.tile([128, 512], f32,
                                                     name="pss_t", tag="pss_t")
                                    nc.tensor.matmul(
                                        pss_t[:, 0:sz],
                                        lhsT=KTh[head][:, k0:k0 + 128],
                                        rhs=QT[p_idx][:, k0 + off:k0 + off + sz],
                                        start=True, stop=True)
                                    if off == 0:
                                        w = min(128, sz)
                                        nc.vector.tensor_add(
                                            pss_t[:, 0:w], pss_t[:, 0:w],
                                            tri[:, 0:w])
                                    nc.scalar.activation(
                                        out=pt[:, off:off + sz],
                                        in_=pss_t[:, 0:sz], func=Exp)
                                    off += 512
                                for qc in range(kt // 4, NQC):
                                    q_lo = max(qc * 512, k0)
                                    n = (qc + 1) * 512 - q_lo
                                    nc.tensor.matmul(
                                        pso_t[qc][0:65, q_lo - qc * 512:512],
                                        lhsT=Vb[kt][:, head * 65:head * 65 + 65],
                                        rhs=pt[:, q_lo - k0:q_lo - k0 + n],
                                        start=(kt == 0), stop=(kt == qc * 4 + 3))
                            for qc in range(NQC):
                                rs0 = work.tile([1, 512], f32, name="rs0", tag="rs0", bufs=1)
                                nc.vector.tensor_copy(rs0, pso_t[qc][64:65, :])
                                rs = work.tile([1, 512], f32, name="rs", tag="rs", bufs=1)
                                nc.vector.reciprocal_approx_fast(out=rs, in_=rs0[0:1, :])
                                rc = work.tile([64, 512], f32, name="rc", tag="rc")
                                nc.gpsimd.partition_broadcast(rc, rs[0:1, :])
                                if part == 0:
                                    nc.vector.tensor_mul(
                                        OT[p_idx][0:64, qc * 512:(qc + 1) * 512],
                                        pso_t[qc][0:64, :], rc)
                                else:
                                    tmp = work.tile([64, 512], f32r, name="tmp", tag="tmp")
                                    nc.vector.tensor_mul(tmp, pso_t[qc][0:64, :], rc)
                                    nc.sync.dma_start(
                                        out=OT[p_idx][64:128, qc * 512:(qc + 1) * 512],
                                        in_=tmp)

                # ---------------- Phase C: output projection ----------------
                with tc.tile_pool(name="psC", bufs=2, space="PSUM") as psC, \
                     tc.tile_pool(name="ypool", bufs=3) as ypool:
                    for tt in range(NT):
                        for half in range(2):
                            psy = psC.tile([128, 512], f32, name="psy", tag="psy")
                            for p in range(4):
                                nc.tensor.matmul(
                                    psy, lhsT=OT[p][:, tt * 128:(tt + 1) * 128],
                                    rhs=wp_sb[p][:, half * 512:(half + 1) * 512],
                                    start=(p == 0), stop=(p == 3))
                            ysb = ypool.tile([128, 512], f32, name="ysb", tag="ysb")
                            nc.vector.tensor_copy(ysb, psy)
                            nc.sync.dma_start(
                                out=y[tt * 128:(tt + 1) * 128,
                                      half * 512:(half + 1) * 512], in_=ysb)

    nc.compile()
    return nc


def _get_nc():
    if "nc" not in _CACHE:
        _CACHE["nc"] = _build()
    return _CACHE["nc"]


def _make_in_maps(x, w_qkv, b_qkv, w_proj):
    scale = np.float32(1.0 / np.sqrt(HEAD_DIM))
    in_maps = []
    for c in range(CORES):
        b = c // 2
        h0 = (c % 2) * HPC
        f0 = h0 * HEAD_DIM
        in_maps.append({
            "xT": np.ascontiguousarray(x[b].T),
            "wq": np.ascontiguousarray(w_qkv[:, f0:f0 + FPC] * scale),
            "wk": np.ascontiguousarray(w_qkv[:, D_MODEL + f0:D_MODEL + f0 + FPC]),
            "wv": np.ascontiguousarray(w_qkv[:, 2 * D_MODEL + f0:2 * D_MODEL + f0 + FPC]),
            "bq": np.ascontiguousarray(
                (b_qkv[f0:f0 + FPC] * scale).reshape(FPC, 1)),
            "wp": np.ascontiguousarray(w_proj[f0:f0 + FPC, :]),
        })
    return in_maps


def kernel(x, w_qkv, b_qkv, w_proj, b_proj):
    from concourse.bass_utils import run_bass_kernel_spmd

    x = np.asarray(x, np.float32)
    w_qkv = np.asarray(w_qkv, np.float32)
    b_qkv = np.asarray(b_qkv, np.float32)
    w_proj = np.asarray(w_proj, np.float32)
    b_proj = np.asarray(b_proj, np.float32)

    nc = _get_nc()
    in_maps = _make_in_maps(x, w_qkv, b_qkv, w_proj)
    res = run_bass_kernel_spmd(nc, in_maps, core_ids=list(range(CORES)))

    # host-side bias corrections: b_proj plus b_v routed through w_proj
    # (softmax rows sum to one, so P @ (1 b_v^T) W_p = 1 (b_v^T W_p))
    bv = b_qkv[2 * D_MODEL:3 * D_MODEL]
    bias_row = (bv @ w_proj + b_proj).astype(np.float32)

    out = np.empty((B, S, D_MODEL), np.float32)
    for b in range(B):
        out[b] = res.results[2 * b]["y"] + res.results[2 * b + 1]["y"] + bias_row
    return out
